# revision 1
# baseline (speedup 1.0000x reference)
"""CapsuleLayer (dynamic routing, 3 iterations) on 8 Trainium2 NeuronCores.

Decomposition (never materializes u_hat = [256,1152,10,16], 189MB):
  - Shard the 1152 input capsules (i) 8 ways: 144 per core.
  - Per-core row space j = (i_local, k), k = in_size = 8 -> 1152 rows
    = 9 chunks of 128 partitions.
  - s_j:  s[b,(n,o)] = sum_j xT[j,b] * (c[j,n] * Wl[j,(n,o)])   (PE matmul,
    contraction over j; Wl = 0.03*W in [(i,k),(n,o)] layout, c broadcast
    over k and o).  Partial over the i-shard -> exchanged across cores.
  - b_ij update via a Gram matrix instead of u_hat:
       Q[j,(n,o)]  = sum_b x[b,j] * v[b,(n,o)]                  (PE matmul)
       pr[j,n]     = sum_o Wl[j,(n,o)] * Q[j,(n,o)]             (DVE)
       uv_rows     = F.T @ pr  per 128-chunk, F = kron(I16, ones8x8)/B
                     (sums over k within each i-group AND replicates the
                     result back to all k-rows, so b stays row-replicated)
  - Iteration 1 uses uniform c = 1/10 (softmax of zeros): s1 = 0.1*(xT.T@Wl).
  - Iterations 1-2 exchange s partials with an fp8-e4m3 AllGather (cheaper
    than AllReduce on this stack) + on-chip tree-reduce; the rounding only
    perturbs the routing weights c_ij (~1e-4 on the final output).
  - Iteration 3 needs no b-update; the final fp32 s3 goes through
    ReduceScatter so each core squashes only its 32-row batch shard; the
    host just concatenates the 8 shards.

Precision plan: routing matmuls use bf16 operands (PSUM accumulates fp32;
fp32 matmuls on trn2 lower to 2x LDWEIGHTS + 2x dual-pass MATMUL, ~8x
slower).  The output-determining iteration-3 matmul uses a 3-product
Dekker split (xtH/xtL, mcH/mcL from an fp32 c3*Wl) so the bf16 PE
reproduces the fp32 result to ~1e-5.  sqrt is a bit-trick + Newton on the
DVE so the ScalarE only ever needs one activation-table set (Exp);
Sqrt/Ln live in other sets and would force ~2.7us ACT_TABLE_LOADs per
iteration.  A tiny warm-up AllGather at kernel start absorbs the one-time
ncfw/collective boot behind the input DMAs and first matmul phase.
"""
import sys

if "/opt/trn_rl_repo" not in sys.path:
    sys.path.insert(0, "/opt/trn_rl_repo")

import numpy as np

import os
N_CORES = int(os.environ.get("KERNEL_CORES", "8"))
B, IN_SIZE, I_TOT = 256, 8, 1152
N_NODE, O_SZ = 10, 16
NO = N_NODE * O_SZ          # 160
I_SH = I_TOT // N_CORES     # 144 capsules per core
JR = I_SH * IN_SIZE         # 1152 rows per core
NCH = JR // 128             # 9 contraction chunks
BC = B // 128               # 2 batch chunks
B_SH = B // N_CORES         # 32 batch rows per core after ReduceScatter

RSQRT_MAGIC = 0x5F3759DF
FAST_S3 = bool(int(os.environ.get("KERNEL_FAST_S3", "0")))

_CACHE = {}


def _build_program():
    import concourse.bacc as bacc
    import concourse.tile as tile
    import concourse.mybir as mybir

    f32 = mybir.dt.float32
    bf16 = mybir.dt.bfloat16
    f8 = mybir.dt.float8e4
    i32 = mybir.dt.int32
    AF = mybir.ActivationFunctionType
    ALU = mybir.AluOpType
    AX = mybir.AxisListType

    nc = bacc.Bacc("TRN2", target_bir_lowering=False, debug=False,
                   enable_asserts=True, num_devices=N_CORES)

    xt_d = nc.dram_tensor("xt", [JR, B], bf16, kind="ExternalInput").ap()
    xik_d = nc.dram_tensor("xik", [B, JR], bf16, kind="ExternalInput").ap()
    wl_d = nc.dram_tensor("wl", [JR, NO], bf16, kind="ExternalInput").ap()
    xtl_d = wlf_d = None
    if not FAST_S3:
        xtl_d = nc.dram_tensor("xtl", [JR, B], bf16,
                               kind="ExternalInput").ap()
        wlf_d = nc.dram_tensor("wlf", [JR, NO], f32,
                               kind="ExternalInput").ap()
    f_d = nc.dram_tensor("fmat", [128, 128], bf16, kind="ExternalInput").ap()
    y_d = nc.dram_tensor("y", [B_SH, NO], f32, kind="ExternalOutput").ap()

    RG = [list(range(N_CORES))]

    with tile.TileContext(nc) as tc:
        with tc.tile_pool(name="persist", bufs=1) as pp, \
             tc.tile_pool(name="work", bufs=1) as wp, \
             tc.tile_pool(name="ps_s", bufs=2, space="PSUM") as ps_s, \
             tc.tile_pool(name="ps_q", bufs=3, space="PSUM") as ps_q, \
             tc.tile_pool(name="ps_f", bufs=1, space="PSUM") as ps_f, \
             tc.tile_pool(name="dram", bufs=1, space="DRAM") as dp:

            # ---------------- input loads ----------------
            xt_sb = pp.tile([128, NCH, B], bf16, name="xt_sb", tag="xt_sb")
            if not FAST_S3:
                xtl_sb = pp.tile([128, NCH, B], bf16, name="xtl_sb",
                                 tag="xtl_sb")
            xik_sb = pp.tile([128, BC, JR], bf16, name="xik_sb", tag="xik_sb")
            wl_sb = pp.tile([128, NCH, NO], bf16, name="wl_sb", tag="wl_sb")
            if not FAST_S3:
                wlf_sb = pp.tile([128, NCH, NO], f32, name="wlf_sb",
                                 tag="wlf_sb")
            f_sb = pp.tile([128, 128], bf16, name="f_sb", tag="f_sb")
            b_sb = pp.tile([128, NCH, N_NODE], f32, name="b_sb", tag="b_sb")

            # Warm-up collective: absorbs the one-time ncfw/TOPSP collective
            # setup (and any cross-core launch skew) concurrently with the
            # input DMAs and the first matmul phase, so the first real
            # AllReduce doesn't pay it on the critical path.
            if int(os.environ.get("KERNEL_WARMUP", "1")):
                warm_in = dp.tile([128, 4], bf16, name="warm_in",
                                  tag="warm_in")
                warm_out = dp.tile([N_CORES * 128, 4], bf16, name="warm_out",
                                   tag="warm_out")
                nc.gpsimd.collective_compute(
                    "AllGather", ALU.bypass, replica_groups=RG,
                    ins=[warm_in.opt()], outs=[warm_out.opt()])

            # Spread input loads across engine DGE queues -- a single issuer
            # serializes ~600ns of descriptor work per DMA.
            engs = [nc.sync, nc.scalar, nc.gpsimd]
            # s1 needs xt+wl first; xik next (Q1); xtl/wlf/F much later.
            xt3 = xt_d.rearrange("(c p) b -> p c b", p=128)
            wl3 = wl_d.rearrange("(c p) f -> p c f", p=128)
            if not FAST_S3:
                xtl3 = xtl_d.rearrange("(c p) b -> p c b", p=128)
                wlf3 = wlf_d.rearrange("(c p) f -> p c f", p=128)
            for g, eng in [((0, 3), nc.sync), ((3, 6), nc.scalar),
                           ((6, NCH), nc.sync)]:
                eng.dma_start(xt_sb[:, g[0]:g[1], :], xt3[:, g[0]:g[1], :])
            for g, eng in [((0, 3), nc.scalar), ((3, 6), nc.sync),
                           ((6, NCH), nc.scalar)]:
                eng.dma_start(wl_sb[:, g[0]:g[1], :], wl3[:, g[0]:g[1], :])
            for bc_i in range(BC):
                engs[bc_i % 2].dma_start(xik_sb[:, bc_i, :],
                                         xik_d[bc_i * 128:(bc_i + 1) * 128, :])
            if not FAST_S3:
                nc.sync.dma_start(xtl_sb[:, 0:5, :], xtl3[:, 0:5, :])
                nc.scalar.dma_start(xtl_sb[:, 5:NCH, :], xtl3[:, 5:NCH, :])
                nc.sync.dma_start(wlf_sb[:, 0:5, :], wlf3[:, 0:5, :])
                nc.scalar.dma_start(wlf_sb[:, 5:NCH, :], wlf3[:, 5:NCH, :])
            nc.sync.dma_start(f_sb[:], f_d[:])

            wl4 = wl_sb[:].rearrange("p c (n o) -> p c n o", n=N_NODE)

            # ---------------- helpers ----------------
            def s_matmul(rhs3, s_sb, scale):
                """s_sb[:,bc,:] = scale * sum_c xt[:,c,bc].T @ rhs3[:,c,:]"""
                for bc_i in range(BC):
                    s_ps = ps_s.tile([128, NO], f32, name="s_ps", tag="s_ps")
                    for c in range(NCH):
                        nc.tensor.matmul(
                            s_ps[:],
                            xt_sb[:, c, bc_i * 128:(bc_i + 1) * 128],
                            rhs3[:, c, :],
                            start=(c == 0), stop=(c == NCH - 1))
                    if scale is None:
                        nc.scalar.copy(s_sb[:, bc_i, :], s_ps[:])
                    else:
                        nc.scalar.mul(s_sb[:, bc_i, :], s_ps[:], scale)

            def allgather_s(s_sb, t):
                """AllGather the bf16 s partials (AG is cheaper than
                AllReduce) and tree-reduce the 8 rank partials on the DVE.
                Payload stays in partition-major [128, BC*NO] layout so every
                DMA is a contiguous 2-D copy. Rounding here only perturbs the
                routing weights c_ij."""
                ag_in = dp.tile([128, BC * NO], f8, name=f"ag_in{t}",
                                tag="ag_in")
                ag_out = dp.tile([N_CORES * 128, BC * NO], f8,
                                 name=f"ag_out{t}", tag="ag_out")
                for bc_i in range(BC):
                    engs[bc_i % 2].dma_start(
                        ag_in[:, bc_i * NO:(bc_i + 1) * NO],
                        s_sb[:, bc_i, :])
                nc.gpsimd.collective_compute(
                    "AllGather", ALU.bypass, replica_groups=RG,
                    ins=[ag_in.opt()], outs=[ag_out.opt()])
                agv = wp.tile([128, N_CORES, BC * NO], f8, name="agv",
                              tag="agv")
                ag3 = ag_out.rearrange("(r p) f -> p r f", p=128)
                nh = N_CORES // 2
                for h in range(nh):
                    engs[h % 3].dma_start(agv[:, 2 * h:2 * h + 2, :],
                                          ag3[:, 2 * h:2 * h + 2, :])
                # leaf adds pair the two ranks of each DMA so the tree starts
                # as soon as individual transfers land
                t4 = wp.tile([128, nh, BC * NO], bf16, name="agt4", tag="agt4")
                for h in range(nh):
                    nc.vector.tensor_add(t4[:, h, :], agv[:, 2 * h, :],
                                         agv[:, 2 * h + 1, :])
                cur = t4[:]
                w = nh
                while w > 2:
                    w //= 2
                    nxt = wp.tile([128, w, BC * NO], bf16,
                                  name=f"agt{w}", tag=f"agt{w}")
                    nc.vector.tensor_add(nxt[:], cur[:, 0:w, :],
                                         cur[:, w:2 * w, :])
                    cur = nxt[:]
                sfull = wp.tile([128, BC, NO], bf16, name="sfull",
                                tag="sfull")
                nc.vector.tensor_add(
                    sfull[:].rearrange("p c f -> p (c f)"),
                    cur[:, 0, :], cur[:, 1, :])
                return sfull

            def rsqrt(msq, P, nch, tag, iters):
                """z ~ 1/sqrt(msq) via int bit-trick + Newton steps (DVE
                only -- avoids the Sqrt/Ln ACT table sets entirely)."""
                sh = [P, nch, N_NODE]
                zi = wp.tile(sh, i32, name="zi" + tag, tag="zi" + tag)
                # zi = ((bits >> 1) ^ -1) + (MAGIC + 1)  ==  MAGIC - (bits>>1)
                nc.vector.tensor_scalar(
                    out=zi[:], in0=msq[:].bitcast(i32), scalar1=1, scalar2=-1,
                    op0=ALU.arith_shift_right, op1=ALU.bitwise_xor)
                nc.vector.tensor_scalar_add(zi[:], zi[:], RSQRT_MAGIC + 1)
                z = zi[:].bitcast(f32)
                t = wp.tile(sh, f32, name="nt" + tag, tag="nt" + tag)
                w = wp.tile(sh, f32, name="nw" + tag, tag="nw" + tag)
                for _ in range(iters):
                    nc.vector.tensor_mul(t[:], z, z)
                    nc.vector.tensor_mul(t[:], t[:], msq[:])
                    nc.vector.tensor_scalar(
                        out=w[:], in0=t[:], scalar1=-0.5, scalar2=1.5,
                        op0=ALU.mult, op1=ALU.add)
                    nc.vector.tensor_mul(z, z, w[:])
                return z

            def squash(s_sb, P, nch, tag, v_dtype, newton_iters=1):
                """v = squash(s) over o. s_sb [P, nch, NO] fp32. One Newton
                step (~0.2% rsqrt error) suffices for the routing iterations;
                the output pass uses two (~1e-5)."""
                s4 = s_sb[:].rearrange("p c (n o) -> p c n o", n=N_NODE)
                sq = wp.tile([P, nch, NO], s_sb.dtype, name="sq" + tag,
                             tag="sq" + tag)
                nc.vector.tensor_mul(sq[:], s_sb[:], s_sb[:])
                msq = wp.tile([P, nch, N_NODE], f32, name="msq" + tag,
                              tag="msq" + tag)
                nc.vector.reduce_sum(
                    msq[:], sq[:].rearrange("p c (n o) -> p c n o", n=N_NODE),
                    axis=AX.X)
                z = rsqrt(msq, P, nch, tag, newton_iters)
                mag = wp.tile([P, nch, N_NODE], f32, name="mag" + tag,
                              tag="mag" + tag)
                nc.vector.tensor_mul(mag[:], msq[:], z)   # sqrt(msq)
                den = wp.tile([P, nch, N_NODE], f32, name="den" + tag,
                              tag="den" + tag)
                nc.vector.tensor_scalar_add(den[:], msq[:], 1.0)
                rden = wp.tile([P, nch, N_NODE], f32, name="rden" + tag,
                               tag="rden" + tag)
                nc.vector.reciprocal(rden[:], den[:])
                fac = wp.tile([P, nch, N_NODE], f32, name="fac" + tag,
                              tag="fac" + tag)
                nc.vector.tensor_mul(fac[:], mag[:], rden[:])
                v_sb = wp.tile([P, nch, NO], v_dtype, name="v" + tag,
                               tag="v" + tag)
                fb = fac[:].unsqueeze(3).broadcast_to((P, nch, N_NODE, O_SZ))
                nc.vector.tensor_mul(
                    v_sb[:].rearrange("p c (n o) -> p c n o", n=N_NODE), s4, fb)
                return v_sb

            def b_update(v_sb, first):
                # Q matmuls pack 3 j-chunks per PSUM bank; p = wl * Q reads
                # each bank straight out of PSUM (3 wide TTs, no Q copies).
                p_sb = wp.tile([128, NCH, NO], bf16, name="p_sb", tag="p_sb")
                for g in range(NCH // 3):
                    q_ps = ps_q.tile([128, 3 * NO], f32, name="q_ps",
                                     tag="q_ps")
                    for s_i in range(3):
                        mc = g * 3 + s_i
                        for bc_i in range(BC):
                            nc.tensor.matmul(
                                q_ps[:, s_i * NO:(s_i + 1) * NO],
                                xik_sb[:, bc_i, mc * 128:(mc + 1) * 128],
                                v_sb[:, bc_i, :],
                                start=(bc_i == 0), stop=(bc_i == BC - 1))
                    nc.vector.tensor_mul(
                        p_sb[:, g * 3:(g + 1) * 3, :],
                        wl_sb[:, g * 3:(g + 1) * 3, :],
                        q_ps[:].rearrange("p (c f) -> p c f", c=3))
                pr = wp.tile([128, NCH, N_NODE], f32, name="pr_sb", tag="pr_sb")
                for g in range(NCH // 3):
                    nc.vector.reduce_sum(
                        pr[:, g * 3:(g + 1) * 3, :],
                        p_sb[:, g * 3:(g + 1) * 3, :].rearrange(
                            "p c (n o) -> p c n o", n=N_NODE),
                        axis=AX.X)
                prb = wp.tile([128, NCH, N_NODE], bf16, name="prb", tag="prb")
                nc.vector.tensor_copy(prb[:], pr[:])
                uv_ps = ps_f.tile([128, NCH * N_NODE], f32, name="uv_ps",
                                  tag="uv_ps")
                nc.tensor.matmul(uv_ps[:], f_sb[:],
                                 prb[:].rearrange("p c n -> p (c n)"),
                                 start=True, stop=True)
                uv3 = uv_ps[:].rearrange("p (c n) -> p c n", n=N_NODE)
                if first:
                    # keep b state for the next update, but let the softmax
                    # read the PSUM uv directly (shorter critical path)
                    nc.scalar.copy(b_sb[:], uv3)
                    return uv3
                nc.vector.tensor_add(b_sb[:], b_sb[:], uv3)
                return b_sb[:]

            def softmax_c(c_dtype, b_src):
                e_sb = wp.tile([128, NCH, N_NODE], f32, name="e_sb", tag="e_sb")
                nc.scalar.activation(e_sb[:], b_src, AF.Exp)
                se = wp.tile([128, NCH], f32, name="se", tag="se")
                nc.vector.reduce_sum(se[:], e_sb[:], axis=AX.X)
                rse = wp.tile([128, NCH], f32, name="rse", tag="rse")
                nc.vector.reciprocal(rse[:], se[:])
                c_sb = wp.tile([128, NCH, N_NODE], c_dtype, name="c_sb",
                               tag="c_sb" + str(c_dtype))
                nc.vector.tensor_mul(
                    c_sb[:], e_sb[:],
                    rse[:].unsqueeze(2).broadcast_to((128, NCH, N_NODE)))
                return c_sb

            def softmax_mc(b_src):
                c_sb = softmax_c(bf16, b_src)
                mc_sb = wp.tile([128, NCH, NO], bf16, name="mc_sb", tag="mc_sb")
                cb = c_sb[:].unsqueeze(3).broadcast_to(
                    (128, NCH, N_NODE, O_SZ))
                mc4 = mc_sb[:].rearrange("p c (n o) -> p c n o", n=N_NODE)
                # split the W-sized multiply across DVE and the idle GpSimd
                nc.vector.tensor_mul(mc4[:, 0:6], wl4[:, 0:6], cb[:, 0:6])
                nc.gpsimd.tensor_mul(mc4[:, 6:NCH], wl4[:, 6:NCH],
                                     cb[:, 6:NCH])
                return mc_sb

            def _dekker_s3(b_src):
                # fp32 c3/mc3, then a 3-product Dekker split so the bf16 PE
                # reproduces the fp32 matmul to ~1e-5:
                #   s3 = xtH.T @ mcH  +  xtH.T @ mcL  +  xtL.T @ mcH
                c3 = softmax_c(f32, b_src)
                mc3 = wp.tile([128, NCH, NO], f32, name="mc3", tag="mc3")
                cb3 = c3[:].unsqueeze(3).broadcast_to(
                    (128, NCH, N_NODE, O_SZ))
                wlf4 = wlf_sb[:].rearrange("p c (n o) -> p c n o", n=N_NODE)
                mc34 = mc3[:].rearrange("p c (n o) -> p c n o", n=N_NODE)
                # hi/lo split, chunk-group-pipelined so the PE can start on
                # early chunks while later ones are still being built
                mcp = wp.tile([128, NCH, 2, NO], bf16, name="mcp", tag="mcp")
                for g in range(NCH // 3):
                    gs = slice(g * 3, (g + 1) * 3)
                    nc.vector.tensor_mul(mc34[:, gs], wlf4[:, gs], cb3[:, gs])
                    nc.scalar.copy(mcp[:, gs, 0, :], mc3[:, gs, :])
                    nc.gpsimd.tensor_sub(mcp[:, gs, 1, :], mc3[:, gs, :],
                                         mcp[:, gs, 0, :])
                s_sb = wp.tile([128, BC, NO], f32, name="s_sb", tag="s_sb")
                for bc_i in range(BC):
                    ps_a = ps_s.tile([128, 2 * NO], f32, name="ps_a",
                                     tag="ps_a")
                    ps_c = ps_s.tile([128, NO], f32, name="s_ps", tag="s_ps")
                    for c in range(NCH):
                        lhs_h = xt_sb[:, c, bc_i * 128:(bc_i + 1) * 128]
                        lhs_l = xtl_sb[:, c, bc_i * 128:(bc_i + 1) * 128]
                        nc.tensor.matmul(
                            ps_a[:], lhs_h,
                            mcp[:, c, :, :].rearrange("p t f -> p (t f)"),
                            start=(c == 0), stop=(c == NCH - 1))
                        nc.tensor.matmul(
                            ps_c[:], lhs_l, mcp[:, c, 0, :],
                            start=(c == 0), stop=(c == NCH - 1))
                    nc.scalar.copy(s_sb[:, bc_i, :], ps_a[:, 0:NO])
                    nc.vector.tensor_add(s_sb[:, bc_i, :], s_sb[:, bc_i, :],
                                         ps_a[:, NO:2 * NO])
                    nc.vector.tensor_add(s_sb[:, bc_i, :], s_sb[:, bc_i, :],
                                         ps_c[:])
                return s_sb

            # ---------------- iteration 1 (c uniform = 0.1) ----------------
            s_sb = wp.tile([128, BC, NO], f8, name="s_sbr", tag="s_sbr")
            s_matmul(wl_sb[:], s_sb, scale=0.1)
            sfull = allgather_s(s_sb, 0)
            v_sb = squash(sfull, 128, BC, "m", bf16)
            b_src = b_update(v_sb, first=True)

            # ---------------- iteration 2 ----------------
            mc_sb = softmax_mc(b_src)
            s_sb = wp.tile([128, BC, NO], f8, name="s_sbr", tag="s_sbr")
            s_matmul(mc_sb[:], s_sb, scale=None)
            sfull = allgather_s(s_sb, 1)
            v_sb = squash(sfull, 128, BC, "m", bf16)
            b_src = b_update(v_sb, first=False)

            # ---------------- iteration 3 (no b-update) ----------------
            if FAST_S3:
                mc_sb = softmax_mc(b_src)
                s_sb = wp.tile([128, BC, NO], f32, name="s_sb", tag="s_sb")
                s_matmul(mc_sb[:], s_sb, scale=None)
            else:
                s_sb = _dekker_s3(b_src)

            rs_in = dp.tile([B, NO], f32, name="rs_in", tag="rs_in")
            rs_out = dp.tile([B_SH, NO], f32, name="rs_out", tag="rs_out")
            for bc_i in range(BC):
                engs[bc_i % 2].dma_start(
                    rs_in[bc_i * 128:(bc_i + 1) * 128, :], s_sb[:, bc_i, :])
            nc.gpsimd.collective_compute(
                "ReduceScatter", ALU.add, replica_groups=RG,
                ins=[rs_in.opt()], outs=[rs_out.opt()])
            ssh = wp.tile([B_SH, 1, NO], f32, name="ssh", tag="ssh")
            nc.sync.dma_start(ssh[:, 0, :], rs_out[:])
            vsh = squash(ssh, B_SH, 1, "s", f32, newton_iters=2)
            nc.sync.dma_start(y_d[:], vsh[:, 0, :])
            rs_in = dp.tile([B, NO], f32, name="rs_in", tag="rs_in")
            rs_out = dp.tile([B_SH, NO], f32, name="rs_out", tag="rs_out")
            for bc_i in range(BC):
                engs[bc_i % 2].dma_start(
                    rs_in[bc_i * 128:(bc_i + 1) * 128, :], s_sb[:, bc_i, :])
            nc.gpsimd.collective_compute(
                "ReduceScatter", ALU.add, replica_groups=RG,
                ins=[rs_in.opt()], outs=[rs_out.opt()])
            ssh = wp.tile([B_SH, 1, NO], f32, name="ssh", tag="ssh")
            nc.sync.dma_start(ssh[:, 0, :], rs_out[:])
            vsh = squash(ssh, B_SH, 1, "s", f32, newton_iters=2)
            nc.sync.dma_start(y_d[:], vsh[:, 0, :])

    nc.compile()
    return nc


def _host_prep(x, W):
    """Per-core input dicts + the constant F matrix."""
    import ml_dtypes

    bf = ml_dtypes.bfloat16
    x = np.ascontiguousarray(x, dtype=np.float32)
    W = np.ascontiguousarray(W, dtype=np.float32)
    F = (np.kron(np.eye(16, dtype=np.float32),
                 np.ones((8, 8), dtype=np.float32)) / np.float32(B)).astype(bf)
    in_maps = []
    for c in range(N_CORES):
        sl = slice(c * I_SH, (c + 1) * I_SH)
        x_sh = x[:, :, sl]                                   # [B, K, I_SH]
        xt = np.ascontiguousarray(x_sh.transpose(2, 1, 0)).reshape(JR, B)
        xt_hi = xt.astype(bf)
        xt_lo = (xt - xt_hi.astype(np.float32)).astype(bf)
        xik = np.ascontiguousarray(
            x_sh.transpose(0, 2, 1)).reshape(B, JR).astype(bf)
        wlf = np.ascontiguousarray(
            (np.float32(0.03) * W[0, sl]).transpose(0, 3, 1, 2)
        ).reshape(JR, NO)
        m = {"xt": xt_hi, "xik": xik, "wl": wlf.astype(bf), "fmat": F}
        if not FAST_S3:
            m["xtl"] = xt_lo
            m["wlf"] = wlf
        in_maps.append(m)
    return in_maps


def _run(in_maps, trace=False, all_cores=False):
    from concourse.bass_utils import run_bass_kernel_spmd

    if "nc" not in _CACHE:
        _CACHE["nc"] = _build_program()
    nc = _CACHE["nc"]
    kwargs = {}
    if all_cores:
        kwargs["trace_cores"] = list(range(N_CORES))
    res = run_bass_kernel_spmd(nc, in_maps, core_ids=list(range(N_CORES)),
                               trace=trace, **kwargs)
    return res


def kernel(x: np.ndarray, W: np.ndarray) -> np.ndarray:
    in_maps = _host_prep(x, W)
    res = _run(in_maps)
    v = np.concatenate([res.results[c]["y"] for c in range(N_CORES)], axis=0)
    return v.reshape(B, N_NODE, O_SZ, 1).astype(np.float32)



# revision 6
# speedup vs baseline: 1.1114x; 1.1114x over previous
"""CapsuleLayer (dynamic routing, 3 iterations) on 8 Trainium2 NeuronCores.

Zero-collective design. The previous AllGather/ReduceScatter kernel spent
~60us waiting for the ncfw/TOPSP collective firmware to boot plus ~10us per
collective; with only ~25us of real math that dominated the runtime. This
version eliminates every collective:

  - The routing state (b_ij, c_ij: [1152,10]) is tiny and the routing is
    statistics-driven (b += mean over 256 batch samples of u_hat.v), so each
    core REPLICATES the full-batch routing (iterations 1-2) instead of
    sharding it. Per-element errors in v average out ~16x in the batch mean,
    so the routing runs in fp8 (measured end-to-end: ~3e-3 final rel err,
    tolerance is 2e-2).
  - Iteration 3 (the only output-determining math) is computed in bf16 with
    each core producing ONLY its 32-row batch shard of v_3. The host
    concatenates the 8 shards: no ReduceScatter, no AllGather, no warm-up.

  Rows j = (i,k) over all 1152 capsules: 9216 rows = 72 chunks of 128.
  Routing matmuls use fp8 DoubleRow perf mode (2 chunks contracted per
  instruction at 0.5 cyc/row = 2x bf16):
    s[b,(n,o)]  = sum_j xt8[j,b] * mc8[j,(n,o)]   36 DoubleRow mm per bc
    Q[j,(n,o)]  = sum_b xik8[b,j] * v8[b,(n,o)]   72 DoubleRow mm (batch
                  contracted 256-deep in one instruction each)
    pr[j,n]     = sum_o wl8 * Q                   (DVE+GpSimd)
    uv          = F.T @ pr  per 128-chunk, F = kron(I16, ones8x8)/2^16
                  (sums over k in each i-group, replicates back to k-rows,
                  and folds the exact 1/(B*SW*SV) = 2^-16 scale)
  Scales: wl8 = 16*0.03*W (keeps fp8 normals), v8 = 16*v; x needs none.
  sqrt via bit-trick + Newton on DVE (only the Exp ACT table is ever
  loaded). Final squash uses 2 Newton steps.
"""
import sys

if "/opt/trn_rl_repo" not in sys.path:
    sys.path.insert(0, "/opt/trn_rl_repo")

import numpy as np

N_CORES = 8
B, IN_SIZE, I_TOT = 256, 8, 1152
N_NODE, O_SZ = 10, 16
NO = N_NODE * O_SZ          # 160
J = I_TOT * IN_SIZE         # 9216 rows (i,k)
NCH = J // 128              # 72 chunks
NG = NCH // 2               # 36 DoubleRow chunk-pairs
B_SH = B // N_CORES         # 32 batch rows per core
SW = 16.0                   # wl fp8 scale
SV = 16.0                   # v fp8 scale
RSQRT_MAGIC = 0x5F3759DF

_CACHE = {}


def _build_program():
    import concourse.bacc as bacc
    import concourse.tile as tile
    import concourse.mybir as mybir

    f32 = mybir.dt.float32
    bf16 = mybir.dt.bfloat16
    f8 = mybir.dt.float8e4
    i32 = mybir.dt.int32
    AF = mybir.ActivationFunctionType
    ALU = mybir.AluOpType
    AX = mybir.AxisListType
    PM = mybir.MatmulPerfMode.DoubleRow

    nc = bacc.Bacc("TRN2", target_bir_lowering=False, debug=False,
                   enable_asserts=True, num_devices=N_CORES)

    xt8_d = nc.dram_tensor("xt8", [J, B], f8, kind="ExternalInput").ap()
    xik8_d = nc.dram_tensor("xik8", [B, J], f8, kind="ExternalInput").ap()
    wl8_d = nc.dram_tensor("wl8", [J, NO], f8, kind="ExternalInput").ap()
    wlb_d = nc.dram_tensor("wlb", [J, NO], bf16, kind="ExternalInput").ap()
    xts_d = nc.dram_tensor("xts", [J, B_SH], bf16, kind="ExternalInput").ap()
    f_d = nc.dram_tensor("fmat", [128, 128], bf16, kind="ExternalInput").ap()
    y_d = nc.dram_tensor("y", [B_SH, NO], f32, kind="ExternalOutput").ap()

    with tile.TileContext(nc) as tc:
        with tc.tile_pool(name="persist", bufs=1) as pp, \
             tc.tile_pool(name="work", bufs=1) as wp, \
             tc.tile_pool(name="pwork", bufs=4) as pw, \
             tc.tile_pool(name="ps_s", bufs=1, space="PSUM") as ps_s, \
             tc.tile_pool(name="ps_q", bufs=3, space="PSUM") as ps_q, \
             tc.tile_pool(name="ps_f", bufs=1, space="PSUM") as ps_f:

            xt8_sb = pp.tile([128, NCH, B], f8, name="xt8_sb", tag="xt8_sb")
            xik8_sb = pp.tile([128, 2, J], f8, name="xik8_sb", tag="xik8_sb")
            wl8_sb = pp.tile([128, NCH, NO], f8, name="wl8_sb", tag="wl8_sb")
            wlb_sb = pp.tile([128, NCH, NO], bf16, name="wlb_sb",
                             tag="wlb_sb")
            xts_sb = pp.tile([128, NCH, B_SH], bf16, name="xts_sb",
                             tag="xts_sb")
            f_sb = pp.tile([128, 128], bf16, name="f_sb", tag="f_sb")
            b_sb = pp.tile([128, NCH, N_NODE], f32, name="b_sb", tag="b_sb")

            # ---------------- input loads ----------------
            # Wave 1 (gates s1): wl8+xt8 interleaved in chunk-order slabs on
            # two DGE queues so the s1 matmul can start on early chunks while
            # later ones stream. Wave 2: xik8 (Q1). Wave 3: wlb+xts (iter 3).
            xt83 = xt8_d.rearrange("(c p) b -> p c b", p=128)
            wl83 = wl8_d.rearrange("(c p) f -> p c f", p=128)
            wlb3 = wlb_d.rearrange("(c p) f -> p c f", p=128)
            xts3 = xts_d.rearrange("(c p) b -> p c b", p=128)
            xik3 = xik8_d.rearrange("(t p) j -> p t j", p=128)
            nc.gpsimd.dma_start(f_sb[:], f_d[:])
            SL = 12
            for si in range(NCH // SL):
                cs = slice(si * SL, (si + 1) * SL)
                nc.sync.dma_start(wl8_sb[:, cs, :], wl83[:, cs, :])
                nc.scalar.dma_start(xt8_sb[:, cs, :], xt83[:, cs, :])
            JQ = J // 4
            for qi in range(4):
                js = slice(qi * JQ, (qi + 1) * JQ)
                [nc.sync, nc.scalar][qi % 2].dma_start(
                    xik8_sb[:, :, js], xik3[:, :, js])
            H = NCH // 2
            nc.sync.dma_start(wlb_sb[:, 0:H, :], wlb3[:, 0:H, :])
            nc.scalar.dma_start(wlb_sb[:, H:NCH, :], wlb3[:, H:NCH, :])
            nc.sync.dma_start(xts_sb[:, 0:H, :], xts3[:, 0:H, :])
            nc.scalar.dma_start(xts_sb[:, H:NCH, :], xts3[:, H:NCH, :])

            wl84 = wl8_sb[:].rearrange("p c (n o) -> p c n o", n=N_NODE)
            wlb4 = wlb_sb[:].rearrange("p c (n o) -> p c n o", n=N_NODE)

            # ---------------- helpers ----------------
            def s_matmul_f8(rhs_sb, s_sb, scale):
                """s_sb[:,bc,:] = scale * sum_j xt8[j,bc].T @ rhs over all
                72 chunks; DoubleRow contracts chunk-pairs."""
                bank = [ps_s.tile([128, NO], f32, name=f"s_ps{bc}",
                                  tag=f"s_ps{bc}") for bc in range(2)]
                for g in range(NG):
                    for bc in range(2):
                        nc.tensor.matmul(
                            bank[bc][:],
                            xt8_sb[:, 2 * g:2 * g + 2,
                                   bc * 128:(bc + 1) * 128],
                            rhs_sb[:, 2 * g:2 * g + 2, :],
                            start=(g == 0), stop=(g == NG - 1),
                            perf_mode=PM)
                for bc in range(2):
                    nc.scalar.mul(s_sb[:, bc, :], bank[bc][:], scale)

            def rsqrt(msq, P, nch, tag, iters):
                sh = [P, nch, N_NODE]
                zi = wp.tile(sh, i32, name="zi" + tag, tag="zi" + tag)
                nc.vector.tensor_scalar(
                    out=zi[:], in0=msq[:].bitcast(i32), scalar1=1, scalar2=-1,
                    op0=ALU.arith_shift_right, op1=ALU.bitwise_xor)
                nc.vector.tensor_scalar_add(zi[:], zi[:], RSQRT_MAGIC + 1)
                z = zi[:].bitcast(f32)
                t = wp.tile(sh, f32, name="nt" + tag, tag="nt" + tag)
                w = wp.tile(sh, f32, name="nw" + tag, tag="nw" + tag)
                for _ in range(iters):
                    nc.vector.tensor_mul(t[:], z, z)
                    nc.vector.tensor_mul(t[:], t[:], msq[:])
                    nc.vector.tensor_scalar(
                        out=w[:], in0=t[:], scalar1=-0.5, scalar2=1.5,
                        op0=ALU.mult, op1=ALU.add)
                    nc.vector.tensor_mul(z, z, w[:])
                return z

            def squash(s_sb, P, nch, tag, v_dtype, fac_scale=None,
                       newton_iters=1):
                """v = squash(s) over o; optional extra scale on the factor
                (for fp8 v8 = SV*v)."""
                s4 = s_sb[:].rearrange("p c (n o) -> p c n o", n=N_NODE)
                sq = wp.tile([P, nch, NO], s_sb.dtype, name="sq" + tag,
                             tag="sq" + tag)
                nc.vector.tensor_mul(sq[:], s_sb[:], s_sb[:])
                msq = wp.tile([P, nch, N_NODE], f32, name="msq" + tag,
                              tag="msq" + tag)
                nc.vector.reduce_sum(
                    msq[:], sq[:].rearrange("p c (n o) -> p c n o", n=N_NODE),
                    axis=AX.X)
                z = rsqrt(msq, P, nch, tag, newton_iters)
                mag = wp.tile([P, nch, N_NODE], f32, name="mag" + tag,
                              tag="mag" + tag)
                nc.vector.tensor_mul(mag[:], msq[:], z)   # sqrt(msq)
                den = wp.tile([P, nch, N_NODE], f32, name="den" + tag,
                              tag="den" + tag)
                nc.vector.tensor_scalar_add(den[:], msq[:], 1.0)
                rden = wp.tile([P, nch, N_NODE], f32, name="rden" + tag,
                               tag="rden" + tag)
                nc.vector.reciprocal(rden[:], den[:])
                fac = wp.tile([P, nch, N_NODE], f32, name="fac" + tag,
                              tag="fac" + tag)
                nc.vector.tensor_mul(fac[:], mag[:], rden[:])
                if fac_scale is not None:
                    nc.vector.tensor_scalar_mul(fac[:], fac[:], fac_scale)
                v_sb = wp.tile([P, nch, NO], v_dtype, name="v" + tag,
                               tag="v" + tag)
                fb = fac[:].unsqueeze(3).broadcast_to((P, nch, N_NODE, O_SZ))
                nc.vector.tensor_mul(
                    v_sb[:].rearrange("p c (n o) -> p c n o", n=N_NODE),
                    s4, fb)
                return v_sb

            def b_update(v8_sb, first):
                """Q per chunk (DoubleRow, 256-deep batch contraction), then
                pr = sum_o wl8*Q (DVE/GpSimd pipelined per 3-chunk PSUM
                bank), then uv = F.T @ pr in 2 halves. Returns exp() input
                views."""
                prb = wp.tile([128, NCH, N_NODE], bf16, name="prb", tag="prb")
                for gq in range(NCH // 3):
                    q_ps = ps_q.tile([128, 3 * NO], f32, name="q_ps",
                                     tag="q_ps")
                    for s_i in range(3):
                        mc = gq * 3 + s_i
                        nc.tensor.matmul(
                            q_ps[:, s_i * NO:(s_i + 1) * NO],
                            xik8_sb[:, :, mc * 128:(mc + 1) * 128],
                            v8_sb[:],
                            start=True, stop=True, perf_mode=PM)
                    p_sb = pw.tile([128, 3, NO], bf16, name="p_sb",
                                   tag="p_sb")
                    nc.vector.tensor_mul(
                        p_sb[:], wl8_sb[:, gq * 3:(gq + 1) * 3, :],
                        q_ps[:].rearrange("p (c f) -> p c f", c=3))
                    with nc.allow_low_precision(
                            reason="pr in bf16 feeds the fp8 routing only; "
                                   "validated at 3e-3 end-to-end"):
                        nc.vector.reduce_sum(
                            prb[:, gq * 3:(gq + 1) * 3, :],
                            p_sb[:].rearrange("p c (n o) -> p c n o",
                                              n=N_NODE),
                            axis=AX.X)
                uvs = []
                for h in range(2):
                    uv_ps = ps_f.tile([128, (NCH // 2) * N_NODE], f32,
                                      name=f"uv_ps{h}", tag=f"uv_ps{h}")
                    nc.tensor.matmul(
                        uv_ps[:], f_sb[:],
                        prb[:, h * (NCH // 2):(h + 1) * (NCH // 2), :]
                        .rearrange("p c n -> p (c n)"),
                        start=True, stop=True)
                    uv3 = uv_ps[:].rearrange("p (c n) -> p c n", n=N_NODE)
                    hs = slice(h * (NCH // 2), (h + 1) * (NCH // 2))
                    if first:
                        nc.scalar.copy(b_sb[:, hs, :], uv3)
                        uvs.append(uv3)
                    else:
                        nc.vector.tensor_add(b_sb[:, hs, :], b_sb[:, hs, :],
                                             uv3)
                        uvs.append(b_sb[:, hs, :])
                return uvs

            def softmax_c(b_srcs):
                e_sb = wp.tile([128, NCH, N_NODE], f32, name="e_sb",
                               tag="e_sb")
                for h in range(2):
                    hs = slice(h * (NCH // 2), (h + 1) * (NCH // 2))
                    nc.scalar.activation(e_sb[:, hs, :], b_srcs[h], AF.Exp)
                se = wp.tile([128, NCH], f32, name="se", tag="se")
                nc.vector.reduce_sum(se[:], e_sb[:], axis=AX.X)
                rse = wp.tile([128, NCH], f32, name="rse", tag="rse")
                nc.vector.reciprocal(rse[:], se[:])
                c_sb = wp.tile([128, NCH, N_NODE], bf16, name="c_sb",
                               tag="c_sb")
                nc.vector.tensor_mul(
                    c_sb[:], e_sb[:],
                    rse[:].unsqueeze(2).broadcast_to((128, NCH, N_NODE)))
                return c_sb

            def build_mc(c_sb, wl4_src, dtype, tag):
                """mc = broadcast(c) * wl, written in chunk slabs so the PE
                can consume early chunks while later ones build. GpSimd takes
                a tail slab to offload the DVE."""
                mc = wp.tile([128, NCH, NO], dtype, name=tag, tag=tag)
                mc4 = mc[:].rearrange("p c (n o) -> p c n o", n=N_NODE)
                cb = c_sb[:].unsqueeze(3).broadcast_to(
                    (128, NCH, N_NODE, O_SZ))
                for sl in range(6):
                    cs = slice(sl * 12, (sl + 1) * 12)
                    eng = nc.gpsimd if sl == 5 else nc.vector
                    eng.tensor_mul(mc4[:, cs], wl4_src[:, cs], cb[:, cs])
                return mc

            # ---------------- iteration 1 (c uniform = 0.1) ----------------
            s_sb = wp.tile([128, 2, NO], f32, name="s_sb1", tag="s_sb")
            s_matmul_f8(wl8_sb[:], s_sb, scale=0.1 / SW)
            v8 = squash(s_sb, 128, 2, "m", f8, fac_scale=SV)
            uvs = b_update(v8, first=True)

            # ---------------- iteration 2 ----------------
            c_sb = softmax_c(uvs)
            mc8 = build_mc(c_sb, wl84, f8, "mc8")
            s_sb = wp.tile([128, 2, NO], f32, name="s_sb2", tag="s_sb")
            s_matmul_f8(mc8[:], s_sb, scale=1.0 / SW)
            v8 = squash(s_sb, 128, 2, "m", f8, fac_scale=SV)
            uvs = b_update(v8, first=False)

            # ---------------- iteration 3: bf16, own batch shard ----------
            c_sb = softmax_c(uvs)
            mc3 = build_mc(c_sb, wlb4, bf16, "mc3")
            s3_ps = ps_s.tile([B_SH, NO], f32, name="s3_ps", tag="s3_ps")
            for c in range(NCH):
                nc.tensor.matmul(s3_ps[:], xts_sb[:, c, :], mc3[:, c, :],
                                 start=(c == 0), stop=(c == NCH - 1))
            ssh = wp.tile([B_SH, 1, NO], f32, name="ssh", tag="ssh")
            nc.scalar.copy(ssh[:, 0, :], s3_ps[:])
            vsh = squash(ssh, B_SH, 1, "s", f32, newton_iters=2)
            nc.sync.dma_start(y_d[:], vsh[:, 0, :])

    nc.compile()
    return nc


def _host_prep(x, W):
    """Per-core input dicts; only xts (the 32-row batch shard of x in bf16)
    differs between cores."""
    import ml_dtypes

    bf = ml_dtypes.bfloat16
    f8 = ml_dtypes.float8_e4m3
    x = np.ascontiguousarray(x, dtype=np.float32)
    W = np.ascontiguousarray(W, dtype=np.float32)
    xt = np.ascontiguousarray(x.transpose(2, 1, 0)).reshape(J, B)
    xik = np.ascontiguousarray(x.transpose(0, 2, 1)).reshape(B, J)
    wl = np.ascontiguousarray(
        (np.float32(0.03) * W[0]).transpose(0, 3, 1, 2)).reshape(J, NO)
    xt8 = xt.astype(f8)
    xik8 = xik.astype(f8)
    wl8 = (wl * np.float32(SW)).astype(f8)
    wlb = wl.astype(bf)
    xtb = xt.astype(bf)
    # F entries 1/(B*SW*SV) = 2^-16: exact in bf16.
    F = (np.kron(np.eye(16, dtype=np.float32),
                 np.ones((8, 8), dtype=np.float32))
         / np.float32(B * SW * SV)).astype(bf)
    base = {"xt8": xt8, "xik8": xik8, "wl8": wl8, "wlb": wlb, "fmat": F}
    in_maps = []
    for c in range(N_CORES):
        m = dict(base)
        m["xts"] = np.ascontiguousarray(
            xtb[:, c * B_SH:(c + 1) * B_SH])
        in_maps.append(m)
    return in_maps


def _run(in_maps, trace=False, all_cores=False):
    from concourse.bass_utils import run_bass_kernel_spmd

    if "nc" not in _CACHE:
        _CACHE["nc"] = _build_program()
    nc = _CACHE["nc"]
    kwargs = {}
    if all_cores:
        kwargs["trace_cores"] = list(range(N_CORES))
    res = run_bass_kernel_spmd(nc, in_maps, core_ids=list(range(N_CORES)),
                               trace=trace, **kwargs)
    return res


def kernel(x: np.ndarray, W: np.ndarray) -> np.ndarray:
    in_maps = _host_prep(x, W)
    res = _run(in_maps)
    v = np.concatenate([res.results[c]["y"] for c in range(N_CORES)], axis=0)
    return v.reshape(B, N_NODE, O_SZ, 1).astype(np.float32)


# revision 9
# speedup vs baseline: 1.2063x; 1.0854x over previous
"""CapsuleLayer (dynamic routing, 3 iterations) on 8 Trainium2 NeuronCores.

Zero-collective design. A collective-based kernel spends ~60us waiting for
the ncfw/TOPSP firmware to boot plus ~10us per collective; with ~25us of
real math that dominates. This kernel eliminates every collective:

  - The routing statistics (b_ij += mean over 256 batch samples of u_hat.v)
    tolerate large per-element noise (it averages out ~16x in the batch
    mean), so each core REPLICATES the full-batch routing (iterations 1-2)
    in fp8 instead of sharding it (measured ~3e-3 final rel err vs the 2e-2
    gate, identical to bf16 routing).
  - Iteration 3 (output-determining) runs in bf16 with each core producing
    only its 32-row batch shard of v_3; the host concatenates. No
    ReduceScatter, no AllGather, no warm-up, no ncfw boot.

Layout: rows j = (i,k), 9216 rows = 72 chunks of 128. All DRAM inputs are
host-packed partition-major ([128, ...] contiguous) so every DMA is a full
2D contiguous transfer (~380 GB/s measured; the naive (c p)->p c gather ran
at ~96 GB/s). One DGE issuer (sync) carries all input waves in priority
order: (wl8,xt8 slabs for s1) -> (xik8+wlb for Q1) -> (xts for s3).

Engine assignment per measured rates (DVE 0.54 ns/el packed-bf16 /
1.07 broadcast or reduce / 1.17 psum-read; GpSimd 1.95; Scalar 1.37 psum
egress; fp8 DoubleRow matmul 69 ns warm = 256-deep contraction per instr):
  s/Q matmuls     fp8 DoubleRow on PE (xt8/xik8 packed [128,2,...])
  Q psum egress   Scalar copy -> bf16 (3 of every 4 groups), DVE direct
                  psum-multiply for the 4th
  p = wlb*Q       DVE packed bf16 multiply over 9-chunk runs
  pr = sum_o p    tree-reduce (16->8->4->2->1), half on DVE, half GpSimd
  uv = F.T @ pr   PE, F = kron(I16, ones8x8)/(B*SV) = 2^-12 exact in bf16
                  (sums k within i-groups, replicates back, folds scales)
  mc = c o wl     broadcast-multiply, slabs split DVE (4) / GpSimd (2),
                  pipelined with the consuming s-matmul
  softmax/squash  ScalarE Exp (table prewarmed) + DVE; sqrt via bit-trick
                  + Newton so only the Exp ACT table is ever loaded.
Scales: wl8 = 16*0.03*W keeps fp8 normals; v8 = 16*v; x unscaled.
"""
import sys

if "/opt/trn_rl_repo" not in sys.path:
    sys.path.insert(0, "/opt/trn_rl_repo")

import numpy as np

N_CORES = 8
B, IN_SIZE, I_TOT = 256, 8, 1152
N_NODE, O_SZ = 10, 16
NO = N_NODE * O_SZ          # 160
J = I_TOT * IN_SIZE         # 9216 rows (i,k)
NCH = J // 128              # 72 chunks
NG = NCH // 2               # 36 DoubleRow chunk-pairs
NH = NCH // 2               # 36 chunks per b_update half
B_SH = B // N_CORES         # 32 batch rows per core
SW = 16.0                   # wl fp8 scale
SV = 16.0                   # v fp8 scale
RSQRT_MAGIC = 0x5F3759DF

_CACHE = {}


def _build_program():
    import concourse.bacc as bacc
    import concourse.tile as tile
    import concourse.mybir as mybir

    f32 = mybir.dt.float32
    bf16 = mybir.dt.bfloat16
    f8 = mybir.dt.float8e4
    i32 = mybir.dt.int32
    AF = mybir.ActivationFunctionType
    ALU = mybir.AluOpType
    AX = mybir.AxisListType
    PM = mybir.MatmulPerfMode.DoubleRow

    nc = bacc.Bacc("TRN2", target_bir_lowering=False, debug=False,
                   enable_asserts=True, num_devices=N_CORES)

    # all inputs host-packed partition-major: one contiguous 2D DMA each
    xt8_d = nc.dram_tensor("xt8", [128, NCH * B], f8,
                           kind="ExternalInput").ap()
    xik8_d = nc.dram_tensor("xik8", [128, 2 * J], f8,
                            kind="ExternalInput").ap()
    wl8_d = nc.dram_tensor("wl8", [128, NCH * NO], f8,
                           kind="ExternalInput").ap()
    wlb_d = nc.dram_tensor("wlb", [128, NCH * NO], bf16,
                           kind="ExternalInput").ap()
    xts_d = nc.dram_tensor("xts", [128, NCH * B_SH], bf16,
                           kind="ExternalInput").ap()
    f_d = nc.dram_tensor("fmat", [128, 128], bf16, kind="ExternalInput").ap()
    y_d = nc.dram_tensor("y", [B_SH, NO], f32, kind="ExternalOutput").ap()

    with tile.TileContext(nc) as tc:
        with tc.tile_pool(name="persist", bufs=1) as pp, \
             tc.tile_pool(name="work", bufs=1) as wp, \
             tc.tile_pool(name="half", bufs=1) as hp, \
             tc.tile_pool(name="ps_s", bufs=1, space="PSUM") as ps_s, \
             tc.tile_pool(name="ps_q", bufs=3, space="PSUM") as ps_q, \
             tc.tile_pool(name="ps_f", bufs=1, space="PSUM") as ps_f:

            xt8_sb = pp.tile([128, NCH, B], f8, name="xt8_sb", tag="xt8_sb")
            xik8_sb = pp.tile([128, 2, J], f8, name="xik8_sb", tag="xik8_sb")
            wl8_sb = pp.tile([128, NCH, NO], f8, name="wl8_sb", tag="wl8_sb")
            wlb_sb = pp.tile([128, NCH, NO], bf16, name="wlb_sb",
                             tag="wlb_sb")
            xts_sb = pp.tile([128, NCH, B_SH], bf16, name="xts_sb",
                             tag="xts_sb")
            f_sb = pp.tile([128, 128], bf16, name="f_sb", tag="f_sb")
            b_sb = pp.tile([128, NCH, N_NODE], f32, name="b_sb", tag="b_sb")

            # ---------------- input loads ----------------
            # One issuer (sync): its DGE serializes the waves in priority
            # order while fanning each transfer across all 16 hw queues.
            xt8f = xt8_sb[:].rearrange("p c b -> p (c b)")
            wl8f = wl8_sb[:].rearrange("p c f -> p (c f)")
            wlbf = wlb_sb[:].rearrange("p c f -> p (c f)")
            xikf = xik8_sb[:].rearrange("p t j -> p (t j)")
            xtsf = xts_sb[:].rearrange("p c b -> p (c b)")
            nc.gpsimd.dma_start(f_sb[:], f_d[:])
            SL = NCH // 4
            for si in range(4):  # wave 1: s1 inputs, chunk-interleaved
                cs = slice(si * SL * NO, (si + 1) * SL * NO)
                nc.sync.dma_start(wl8f[:, cs], wl8_d[:, cs])
                cs = slice(si * SL * B, (si + 1) * SL * B)
                nc.sync.dma_start(xt8f[:, cs], xt8_d[:, cs])
            for qi in range(4):  # wave 2: Q1 inputs
                js = slice(qi * J // 2, (qi + 1) * J // 2)
                nc.sync.dma_start(xikf[:, js], xik8_d[:, js])
                ws = slice(qi * NCH // 4 * NO, (qi + 1) * NCH // 4 * NO)
                nc.sync.dma_start(wlbf[:, ws], wlb_d[:, ws])
            nc.sync.dma_start(xtsf[:], xts_d[:])  # wave 3: iter-3 input

            # prewarm the Exp ACT table during the DMA wait
            warm = wp.tile([128, 1], f32, name="warm", tag="warm")
            nc.vector.memset(warm[:], 0.0)
            nc.scalar.activation(warm[:], warm[:], AF.Exp)

            wl84 = wl8_sb[:].rearrange("p c (n o) -> p c n o", n=N_NODE)
            wlb4 = wlb_sb[:].rearrange("p c (n o) -> p c n o", n=N_NODE)

            # ---------------- helpers ----------------
            def s_matmul_f8(rhs_sb, s_sb, scale):
                """s_sb[:,bc,:] = scale * sum over all 72 chunks (DoubleRow
                chunk-pairs) of xt8.T @ rhs."""
                bank = [ps_s.tile([128, NO], f32, name=f"s_ps{bc}",
                                  tag=f"s_ps{bc}") for bc in range(2)]
                for g in range(NG):
                    for bc in range(2):
                        nc.tensor.matmul(
                            bank[bc][:],
                            xt8_sb[:, 2 * g:2 * g + 2,
                                   bc * 128:(bc + 1) * 128],
                            rhs_sb[:, 2 * g:2 * g + 2, :],
                            start=(g == 0), stop=(g == NG - 1),
                            perf_mode=PM)
                for bc in range(2):
                    nc.scalar.mul(s_sb[:, bc, :], bank[bc][:], scale)

            def rsqrt(msq, P, nch, tag, iters):
                sh = [P, nch, N_NODE]
                zi = wp.tile(sh, i32, name="zi" + tag, tag="zi" + tag)
                nc.vector.tensor_scalar(
                    out=zi[:], in0=msq[:].bitcast(i32), scalar1=1, scalar2=-1,
                    op0=ALU.arith_shift_right, op1=ALU.bitwise_xor)
                nc.vector.tensor_scalar_add(zi[:], zi[:], RSQRT_MAGIC + 1)
                z = zi[:].bitcast(f32)
                t = wp.tile(sh, f32, name="nt" + tag, tag="nt" + tag)
                w = wp.tile(sh, f32, name="nw" + tag, tag="nw" + tag)
                for _ in range(iters):
                    nc.vector.tensor_mul(t[:], z, z)
                    nc.vector.tensor_mul(t[:], t[:], msq[:])
                    nc.vector.tensor_scalar(
                        out=w[:], in0=t[:], scalar1=-0.5, scalar2=1.5,
                        op0=ALU.mult, op1=ALU.add)
                    nc.vector.tensor_mul(z, z, w[:])
                return z

            def squash(s_sb, P, nch, tag, v_dtype, fac_scale=None,
                       newton_iters=1):
                s4 = s_sb[:].rearrange("p c (n o) -> p c n o", n=N_NODE)
                sq = wp.tile([P, nch, NO], s_sb.dtype, name="sq" + tag,
                             tag="sq" + tag)
                nc.vector.tensor_mul(sq[:], s_sb[:], s_sb[:])
                msq = wp.tile([P, nch, N_NODE], f32, name="msq" + tag,
                              tag="msq" + tag)
                nc.vector.reduce_sum(
                    msq[:], sq[:].rearrange("p c (n o) -> p c n o", n=N_NODE),
                    axis=AX.X)
                z = rsqrt(msq, P, nch, tag, newton_iters)
                mag = wp.tile([P, nch, N_NODE], f32, name="mag" + tag,
                              tag="mag" + tag)
                nc.vector.tensor_mul(mag[:], msq[:], z)   # sqrt(msq)
                den = wp.tile([P, nch, N_NODE], f32, name="den" + tag,
                              tag="den" + tag)
                nc.vector.tensor_scalar_add(den[:], msq[:], 1.0)
                rden = wp.tile([P, nch, N_NODE], f32, name="rden" + tag,
                               tag="rden" + tag)
                nc.vector.reciprocal(rden[:], den[:])
                fac = wp.tile([P, nch, N_NODE], f32, name="fac" + tag,
                              tag="fac" + tag)
                nc.vector.tensor_mul(fac[:], mag[:], rden[:])
                if fac_scale is not None:
                    nc.vector.tensor_scalar_mul(fac[:], fac[:], fac_scale)
                v_sb = wp.tile([P, nch, NO], v_dtype, name="v" + tag,
                               tag="v" + tag)
                fb = fac[:].unsqueeze(3).broadcast_to((P, nch, N_NODE, O_SZ))
                nc.vector.tensor_mul(
                    v_sb[:].rearrange("p c (n o) -> p c n o", n=N_NODE),
                    s4, fb)
                return v_sb

            def half_tree(eng, ph4, prb, h):
                """pr[:, half, :] = sum_o p via packed bf16 adds 16->1."""
                t8 = hp.tile([128, NH, N_NODE, 8], bf16, name="t8",
                             tag="t8" + str(h % 2))
                eng.tensor_add(t8[:], ph4[..., 0:8], ph4[..., 8:16])
                t4 = hp.tile([128, NH, N_NODE, 4], bf16, name="t4",
                             tag="t4" + str(h % 2))
                eng.tensor_add(t4[:], t8[:, :, :, 0:4], t8[:, :, :, 4:8])
                t2 = hp.tile([128, NH, N_NODE, 2], bf16, name="t2",
                             tag="t2" + str(h % 2))
                eng.tensor_add(t2[:], t4[:, :, :, 0:2], t4[:, :, :, 2:4])
                eng.tensor_add(
                    prb[:, h * NH:(h + 1) * NH, :].unsqueeze(3),
                    t2[:, :, :, 0:1], t2[:, :, :, 1:2])

            def b_update(v8_sb, first):
                """Q (DoubleRow, 256-deep batch contraction per chunk), then
                pr = sum_o wlb*Q, then uv = F.T @ pr per half. Scalar does
                the PSUM egress for 3 of 4 groups (DVE direct-mults the 4th);
                DVE packed-multiplies; trees split DVE/GpSimd."""
                prb = wp.tile([128, NCH, N_NODE], bf16, name="prb", tag="prb")
                uvs = []
                for h in range(2):
                    ph = hp.tile([128, NH, NO], bf16, name="ph",
                                 tag="ph" + str(h % 2))
                    for r in range(3):  # 3 runs of (3 copied + 1 direct) grp
                        qrun = hp.tile([128, 9, NO], bf16, name="qrun",
                                       tag="qr" + str(r % 2))
                        for gi in range(4):
                            gq = h * 12 + r * 4 + gi
                            q_ps = ps_q.tile([128, 3 * NO], f32, name="q_ps",
                                             tag="q_ps")
                            for s_i in range(3):
                                mc = gq * 3 + s_i
                                nc.tensor.matmul(
                                    q_ps[:, s_i * NO:(s_i + 1) * NO],
                                    xik8_sb[:, :, mc * 128:(mc + 1) * 128],
                                    v8_sb[:],
                                    start=True, stop=True, perf_mode=PM)
                            q3 = q_ps[:].rearrange("p (c f) -> p c f", c=3)
                            lo = (r * 4 + gi) * 3
                            if gi == 3:
                                nc.vector.tensor_mul(
                                    ph[:, lo:lo + 3, :],
                                    wlb_sb[:, h * NH + lo:h * NH + lo + 3, :],
                                    q3)
                            else:
                                nc.scalar.copy(qrun[:, gi * 3:gi * 3 + 3, :],
                                               q3)
                        lo = r * 12
                        nc.vector.tensor_mul(
                            ph[:, lo:lo + 9, :],
                            wlb_sb[:, h * NH + lo:h * NH + lo + 9, :],
                            qrun[:])
                    ph4 = ph[:].rearrange("p c (n o) -> p c n o", n=N_NODE)
                    half_tree(nc.vector if h == 0 else nc.gpsimd,
                              ph4, prb, h)
                    uv_ps = ps_f.tile([128, NH * N_NODE], f32,
                                      name=f"uv_ps{h}", tag=f"uv_ps{h}")
                    nc.tensor.matmul(
                        uv_ps[:], f_sb[:],
                        prb[:, h * NH:(h + 1) * NH, :]
                        .rearrange("p c n -> p (c n)"),
                        start=True, stop=True)
                    uv3 = uv_ps[:].rearrange("p (c n) -> p c n", n=N_NODE)
                    hs = slice(h * NH, (h + 1) * NH)
                    if first:
                        nc.scalar.copy(b_sb[:, hs, :], uv3)
                        uvs.append(uv3)
                    else:
                        nc.vector.tensor_add(b_sb[:, hs, :], b_sb[:, hs, :],
                                             uv3)
                        uvs.append(b_sb[:, hs, :])
                return uvs

            def softmax_c(b_srcs):
                e_sb = wp.tile([128, NCH, N_NODE], f32, name="e_sb",
                               tag="e_sb")
                for h in range(2):
                    hs = slice(h * NH, (h + 1) * NH)
                    nc.scalar.activation(e_sb[:, hs, :], b_srcs[h], AF.Exp)
                se = wp.tile([128, NCH], f32, name="se", tag="se")
                nc.vector.reduce_sum(se[:], e_sb[:], axis=AX.X)
                rse = wp.tile([128, NCH], f32, name="rse", tag="rse")
                nc.vector.reciprocal(rse[:], se[:])
                c_sb = wp.tile([128, NCH, N_NODE], bf16, name="c_sb",
                               tag="c_sb")
                nc.vector.tensor_mul(
                    c_sb[:], e_sb[:],
                    rse[:].unsqueeze(2).broadcast_to((128, NCH, N_NODE)))
                return c_sb

            def build_mc(c_sb, wl4_src, dtype, tag):
                """mc = broadcast(c) * wl in 6 chunk-slabs: DVE takes the
                first 4 (PE consumes in chunk order), GpSimd the last 2."""
                mc = wp.tile([128, NCH, NO], dtype, name=tag, tag=tag)
                mc4 = mc[:].rearrange("p c (n o) -> p c n o", n=N_NODE)
                cb = c_sb[:].unsqueeze(3).broadcast_to(
                    (128, NCH, N_NODE, O_SZ))
                for sl in range(6):
                    cs = slice(sl * 12, (sl + 1) * 12)
                    eng = nc.vector if sl < 4 else nc.gpsimd
                    eng.tensor_mul(mc4[:, cs], wl4_src[:, cs], cb[:, cs])
                return mc

            # ---------------- iteration 1 (c uniform = 0.1) ----------------
            s_sb = wp.tile([128, 2, NO], f32, name="s_sb1", tag="s_sb")
            s_matmul_f8(wl8_sb[:], s_sb, scale=0.1 / SW)
            v8 = squash(s_sb, 128, 2, "m", f8, fac_scale=SV)
            uvs = b_update(v8, first=True)

            # ---------------- iteration 2 ----------------
            c_sb = softmax_c(uvs)
            mc8 = build_mc(c_sb, wl84, f8, "mc8")
            s_sb = wp.tile([128, 2, NO], f32, name="s_sb2", tag="s_sb")
            s_matmul_f8(mc8[:], s_sb, scale=1.0 / SW)
            v8 = squash(s_sb, 128, 2, "m", f8, fac_scale=SV)
            uvs = b_update(v8, first=False)

            # ---------------- iteration 3: bf16, own batch shard ----------
            c_sb = softmax_c(uvs)
            mc3 = build_mc(c_sb, wlb4, bf16, "mc3")
            s3_ps = ps_s.tile([B_SH, NO], f32, name="s3_ps", tag="s3_ps")
            for c in range(NCH):
                nc.tensor.matmul(s3_ps[:], xts_sb[:, c, :], mc3[:, c, :],
                                 start=(c == 0), stop=(c == NCH - 1))
            ssh = wp.tile([B_SH, 1, NO], f32, name="ssh", tag="ssh")
            nc.scalar.copy(ssh[:, 0, :], s3_ps[:])
            vsh = squash(ssh, B_SH, 1, "s", f32, newton_iters=2)
            nc.sync.dma_start(y_d[:], vsh[:, 0, :])

    nc.compile()
    return nc


def _pack_pm(arr2d, cols):
    """[J, cols] row-major -> [128, NCH*cols] partition-major contiguous."""
    return np.ascontiguousarray(
        arr2d.reshape(NCH, 128, cols).transpose(1, 0, 2).reshape(
            128, NCH * cols))


def _host_prep(x, W):
    """Per-core input dicts; only xts (the 32-col batch shard of x, bf16)
    differs between cores."""
    import ml_dtypes

    bf = ml_dtypes.bfloat16
    f8 = ml_dtypes.float8_e4m3
    x = np.ascontiguousarray(x, dtype=np.float32)
    W = np.ascontiguousarray(W, dtype=np.float32)
    xt = np.ascontiguousarray(x.transpose(2, 1, 0)).reshape(J, B)
    xik = np.ascontiguousarray(x.transpose(0, 2, 1)).reshape(B, J)
    wl = np.ascontiguousarray(
        (np.float32(0.03) * W[0]).transpose(0, 3, 1, 2)).reshape(J, NO)
    xt8 = _pack_pm(xt.astype(f8), B)
    xik8 = np.ascontiguousarray(
        xik.astype(f8).reshape(2, 128, J).transpose(1, 0, 2).reshape(
            128, 2 * J))
    wl8 = _pack_pm((wl * np.float32(SW)).astype(f8), NO)
    wlb = _pack_pm(wl.astype(bf), NO)
    xtb = xt.astype(bf)
    # F entries 1/(B*SV) = 2^-12: exact in bf16.
    F = (np.kron(np.eye(16, dtype=np.float32),
                 np.ones((8, 8), dtype=np.float32))
         / np.float32(B * SV)).astype(bf)
    base = {"xt8": xt8, "xik8": xik8, "wl8": wl8, "wlb": wlb, "fmat": F}
    in_maps = []
    for c in range(N_CORES):
        m = dict(base)
        m["xts"] = _pack_pm(np.ascontiguousarray(
            xtb[:, c * B_SH:(c + 1) * B_SH]), B_SH)
        in_maps.append(m)
    return in_maps


def _run(in_maps, trace=False, all_cores=False):
    from concourse.bass_utils import run_bass_kernel_spmd

    if "nc" not in _CACHE:
        _CACHE["nc"] = _build_program()
    nc = _CACHE["nc"]
    kwargs = {}
    if all_cores:
        kwargs["trace_cores"] = list(range(N_CORES))
    res = run_bass_kernel_spmd(nc, in_maps, core_ids=list(range(N_CORES)),
                               trace=trace, **kwargs)
    return res


def kernel(x: np.ndarray, W: np.ndarray) -> np.ndarray:
    in_maps = _host_prep(x, W)
    res = _run(in_maps)
    v = np.concatenate([res.results[c]["y"] for c in range(N_CORES)], axis=0)
    return v.reshape(B, N_NODE, O_SZ, 1).astype(np.float32)


# revision 12
# speedup vs baseline: 1.3592x; 1.1267x over previous
"""CapsuleLayer (dynamic routing, 3 iterations) on 8 Trainium2 NeuronCores.

Zero-collective design. A collective-based kernel spends ~60us waiting for
the ncfw/TOPSP firmware to boot plus ~10us per collective; with ~25us of
real math that dominates. This kernel eliminates every collective:

  - The routing statistics (b_ij += mean over 256 batch samples of u_hat.v)
    tolerate large per-element noise (it averages out ~16x in the batch
    mean), so each core REPLICATES the full-batch routing (iterations 1-2)
    in fp8 instead of sharding it (measured ~3e-3 final rel err vs the 2e-2
    gate, identical to bf16 routing).
  - Iteration 3 (output-determining) runs in bf16 with each core producing
    only its 32-row batch shard of v_3; the host concatenates. No
    ReduceScatter, no AllGather, no warm-up, no ncfw boot.

Layout: rows j = (i,k), 9216 rows = 72 chunks of 128. All DRAM inputs are
host-packed partition-major ([128, ...] contiguous) so every DMA is a full
2D contiguous transfer (~380 GB/s measured; the naive (c p)->p c gather ran
at ~96 GB/s). One DGE issuer (sync) carries all input waves in priority
order: (wl8,xt8 slabs for s1) -> (xik8+wlb for Q1) -> (xts for s3).

Engine assignment per measured rates (DVE 0.54 ns/el packed-bf16 /
1.07 broadcast or reduce / 1.17 psum-read; GpSimd 1.95; Scalar 1.37 psum
egress; fp8 DoubleRow matmul 69 ns warm = 256-deep contraction per instr):
  s/Q matmuls     fp8 DoubleRow on PE (xt8/xik8 packed [128,2,...])
  Q psum egress   Scalar copy -> bf16 (3 of every 4 groups), DVE direct
                  psum-multiply for the 4th
  p = wlb*Q       DVE packed bf16 multiply over 9-chunk runs
  pr = sum_o p    tree-reduce (16->8->4->2->1), half on DVE, half GpSimd
  uv = F.T @ pr   PE, F = kron(I16, ones8x8)/(B*SV) = 2^-12 exact in bf16
                  (sums k within i-groups, replicates back, folds scales)
  mc = c o wl     broadcast-multiply, slabs split DVE (4) / GpSimd (2),
                  pipelined with the consuming s-matmul
  softmax/squash  ScalarE Exp (table prewarmed) + DVE; sqrt via bit-trick
                  + Newton so only the Exp ACT table is ever loaded.
Scales: wl8 = 16*0.03*W keeps fp8 normals; v8 = 16*v; x unscaled.
"""
import sys

if "/opt/trn_rl_repo" not in sys.path:
    sys.path.insert(0, "/opt/trn_rl_repo")

import numpy as np

N_CORES = 8
B, IN_SIZE, I_TOT = 256, 8, 1152
N_NODE, O_SZ = 10, 16
NO = N_NODE * O_SZ          # 160
J = I_TOT * IN_SIZE         # 9216 rows (i,k)
NCH = J // 128              # 72 chunks
NG = NCH // 2               # 36 DoubleRow chunk-pairs
NH = NCH // 2               # 36 chunks per b_update half
B_SH = B // N_CORES         # 32 batch rows per core
SW = 16.0                   # wl fp8 scale
SV = 16.0                   # v fp8 scale
RSQRT_MAGIC = 0x5F3759DF

_CACHE = {}


def _build_program():
    import concourse.bacc as bacc
    import concourse.tile as tile
    import concourse.mybir as mybir

    f32 = mybir.dt.float32
    bf16 = mybir.dt.bfloat16
    f8 = mybir.dt.float8e4
    i32 = mybir.dt.int32
    AF = mybir.ActivationFunctionType
    ALU = mybir.AluOpType
    AX = mybir.AxisListType
    PM = mybir.MatmulPerfMode.DoubleRow

    nc = bacc.Bacc("TRN2", target_bir_lowering=False, debug=False,
                   enable_asserts=True, num_devices=N_CORES)

    # all inputs host-packed partition-major: one contiguous 2D DMA each
    xt8_d = nc.dram_tensor("xt8", [128, NCH * B], f8,
                           kind="ExternalInput").ap()
    xik8_d = nc.dram_tensor("xik8", [128, 2 * J], f8,
                            kind="ExternalInput").ap()
    wl8_d = nc.dram_tensor("wl8", [128, NCH * NO], f8,
                           kind="ExternalInput").ap()
    wlb_d = nc.dram_tensor("wlb", [128, NCH * NO], bf16,
                           kind="ExternalInput").ap()
    xts_d = nc.dram_tensor("xts", [128, NCH * B_SH], bf16,
                           kind="ExternalInput").ap()
    f_d = nc.dram_tensor("fmat", [128, 128], bf16, kind="ExternalInput").ap()
    y_d = nc.dram_tensor("y", [B_SH, NO], f32, kind="ExternalOutput").ap()

    with tile.TileContext(nc) as tc:
        with tc.tile_pool(name="persist", bufs=1) as pp, \
             tc.tile_pool(name="work", bufs=1) as wp, \
             tc.tile_pool(name="half", bufs=1) as hp, \
             tc.tile_pool(name="ps_s", bufs=1, space="PSUM") as ps_s, \
             tc.tile_pool(name="ps_q", bufs=3, space="PSUM") as ps_q, \
             tc.tile_pool(name="ps_f", bufs=1, space="PSUM") as ps_f:

            xt8_sb = pp.tile([128, NCH, B], f8, name="xt8_sb", tag="xt8_sb")
            xik8_sb = pp.tile([128, 2, J], f8, name="xik8_sb", tag="xik8_sb")
            wl8_sb = pp.tile([128, NCH, NO], f8, name="wl8_sb", tag="wl8_sb")
            wlb_sb = pp.tile([128, NCH, NO], bf16, name="wlb_sb",
                             tag="wlb_sb")
            xts_sb = pp.tile([128, NCH, B_SH], bf16, name="xts_sb",
                             tag="xts_sb")
            f_sb = pp.tile([128, 128], bf16, name="f_sb", tag="f_sb")
            b_sb = pp.tile([128, NCH, N_NODE], f32, name="b_sb", tag="b_sb")

            # ---------------- input loads ----------------
            # One issuer (sync): its DGE serializes the waves in priority
            # order while fanning each transfer across all 16 hw queues.
            xt8f = xt8_sb[:].rearrange("p c b -> p (c b)")
            wl8f = wl8_sb[:].rearrange("p c f -> p (c f)")
            wlbf = wlb_sb[:].rearrange("p c f -> p (c f)")
            xikf = xik8_sb[:].rearrange("p t j -> p (t j)")
            xtsf = xts_sb[:].rearrange("p c b -> p (c b)")
            nc.gpsimd.dma_start(f_sb[:], f_d[:])
            SL = NCH // 4
            for si in range(4):  # wave 1: s1 inputs, chunk-interleaved
                cs = slice(si * SL * NO, (si + 1) * SL * NO)
                nc.sync.dma_start(wl8f[:, cs], wl8_d[:, cs])
                cs = slice(si * SL * B, (si + 1) * SL * B)
                nc.sync.dma_start(xt8f[:, cs], xt8_d[:, cs])
            for qi in range(4):  # wave 2: Q1 inputs
                js = slice(qi * J // 2, (qi + 1) * J // 2)
                nc.sync.dma_start(xikf[:, js], xik8_d[:, js])
                ws = slice(qi * NCH // 4 * NO, (qi + 1) * NCH // 4 * NO)
                nc.sync.dma_start(wlbf[:, ws], wlb_d[:, ws])
            nc.sync.dma_start(xtsf[:], xts_d[:])  # wave 3: iter-3 input

            # prewarm the Exp ACT table during the DMA wait
            warm = wp.tile([128, 1], f32, name="warm", tag="warm")
            nc.vector.memset(warm[:], 0.0)
            nc.scalar.activation(warm[:], warm[:], AF.Exp)

            wl84 = wl8_sb[:].rearrange("p c (n o) -> p c n o", n=N_NODE)
            wlb4 = wlb_sb[:].rearrange("p c (n o) -> p c n o", n=N_NODE)

            # ---------------- helpers ----------------
            def s_matmul_f8(rhs_sb, s_sb, scale):
                """s_sb[:,bc,:] = scale * sum over all 72 chunks (DoubleRow
                chunk-pairs) of xt8.T @ rhs."""
                bank = [ps_s.tile([128, NO], f32, name=f"s_ps{bc}",
                                  tag=f"s_ps{bc}") for bc in range(2)]
                for g in range(NG):
                    for bc in range(2):
                        nc.tensor.matmul(
                            bank[bc][:],
                            xt8_sb[:, 2 * g:2 * g + 2,
                                   bc * 128:(bc + 1) * 128],
                            rhs_sb[:, 2 * g:2 * g + 2, :],
                            start=(g == 0), stop=(g == NG - 1),
                            perf_mode=PM)
                for bc in range(2):
                    nc.scalar.mul(s_sb[:, bc, :], bank[bc][:], scale)

            def rsqrt(msq, P, nch, tag, iters):
                sh = [P, nch, N_NODE]
                zi = wp.tile(sh, i32, name="zi" + tag, tag="zi" + tag)
                nc.vector.tensor_scalar(
                    out=zi[:], in0=msq[:].bitcast(i32), scalar1=1, scalar2=-1,
                    op0=ALU.arith_shift_right, op1=ALU.bitwise_xor)
                nc.vector.tensor_scalar_add(zi[:], zi[:], RSQRT_MAGIC + 1)
                z = zi[:].bitcast(f32)
                t = wp.tile(sh, f32, name="nt" + tag, tag="nt" + tag)
                w = wp.tile(sh, f32, name="nw" + tag, tag="nw" + tag)
                for _ in range(iters):
                    nc.vector.tensor_mul(t[:], z, z)
                    nc.vector.tensor_mul(t[:], t[:], msq[:])
                    nc.vector.tensor_scalar(
                        out=w[:], in0=t[:], scalar1=-0.5, scalar2=1.5,
                        op0=ALU.mult, op1=ALU.add)
                    nc.vector.tensor_mul(z, z, w[:])
                return z

            def squash(s_sb, P, nch, tag, v_dtype, fac_scale=None,
                       newton_iters=1):
                s4 = s_sb[:].rearrange("p c (n o) -> p c n o", n=N_NODE)
                sq = wp.tile([P, nch, NO], s_sb.dtype, name="sq" + tag,
                             tag="sq" + tag)
                nc.vector.tensor_mul(sq[:], s_sb[:], s_sb[:])
                msq = wp.tile([P, nch, N_NODE], f32, name="msq" + tag,
                              tag="msq" + tag)
                nc.vector.reduce_sum(
                    msq[:], sq[:].rearrange("p c (n o) -> p c n o", n=N_NODE),
                    axis=AX.X)
                z = rsqrt(msq, P, nch, tag, newton_iters)
                mag = wp.tile([P, nch, N_NODE], f32, name="mag" + tag,
                              tag="mag" + tag)
                nc.vector.tensor_mul(mag[:], msq[:], z)   # sqrt(msq)
                den = wp.tile([P, nch, N_NODE], f32, name="den" + tag,
                              tag="den" + tag)
                nc.vector.tensor_scalar_add(den[:], msq[:], 1.0)
                rden = wp.tile([P, nch, N_NODE], f32, name="rden" + tag,
                               tag="rden" + tag)
                nc.vector.reciprocal(rden[:], den[:])
                fac = wp.tile([P, nch, N_NODE], f32, name="fac" + tag,
                              tag="fac" + tag)
                nc.vector.tensor_mul(fac[:], mag[:], rden[:])
                if fac_scale is not None:
                    nc.vector.tensor_scalar_mul(fac[:], fac[:], fac_scale)
                v_sb = wp.tile([P, nch, NO], v_dtype, name="v" + tag,
                               tag="v" + tag)
                fb = fac[:].unsqueeze(3).broadcast_to((P, nch, N_NODE, O_SZ))
                nc.vector.tensor_mul(
                    v_sb[:].rearrange("p c (n o) -> p c n o", n=N_NODE),
                    s4, fb)
                return v_sb

            def half_tree(eng, ph4, prb, h):
                """pr[:, half, :] = sum_o p via packed bf16 adds 16->1."""
                t8 = hp.tile([128, NH, N_NODE, 8], bf16, name="t8",
                             tag="t8" + str(h % 2))
                eng.tensor_add(t8[:], ph4[..., 0:8], ph4[..., 8:16])
                t4 = hp.tile([128, NH, N_NODE, 4], bf16, name="t4",
                             tag="t4" + str(h % 2))
                eng.tensor_add(t4[:], t8[:, :, :, 0:4], t8[:, :, :, 4:8])
                t2 = hp.tile([128, NH, N_NODE, 2], bf16, name="t2",
                             tag="t2" + str(h % 2))
                eng.tensor_add(t2[:], t4[:, :, :, 0:2], t4[:, :, :, 2:4])
                eng.tensor_add(
                    prb[:, h * NH:(h + 1) * NH, :].unsqueeze(3),
                    t2[:, :, :, 0:1], t2[:, :, :, 1:2])

            def b_update(v8_sb, first):
                """Q (DoubleRow, 256-deep batch contraction per chunk), then
                pr = sum_o wlb*Q, then uv = F.T @ pr per half. Scalar does
                the PSUM egress for 3 of 4 groups (DVE direct-mults the 4th);
                DVE packed-multiplies; trees split DVE/GpSimd."""
                prb = wp.tile([128, NCH, N_NODE], bf16, name="prb", tag="prb")
                uvs = []
                for h in range(2):
                    ph = hp.tile([128, NH, NO], bf16, name="ph",
                                 tag="ph" + str(h % 2))
                    for r in range(3):  # 3 runs of (3 copied + 1 direct) grp
                        qrun = hp.tile([128, 9, NO], bf16, name="qrun",
                                       tag="qr" + str(r % 2))
                        for gi in range(4):
                            gq = h * 12 + r * 4 + gi
                            q_ps = ps_q.tile([128, 3 * NO], f32, name="q_ps",
                                             tag="q_ps")
                            for s_i in range(3):
                                mc = gq * 3 + s_i
                                nc.tensor.matmul(
                                    q_ps[:, s_i * NO:(s_i + 1) * NO],
                                    xik8_sb[:, :, mc * 128:(mc + 1) * 128],
                                    v8_sb[:],
                                    start=True, stop=True, perf_mode=PM)
                            q3 = q_ps[:].rearrange("p (c f) -> p c f", c=3)
                            lo = (r * 4 + gi) * 3
                            if gi == 3:
                                nc.vector.tensor_mul(
                                    ph[:, lo:lo + 3, :],
                                    wlb_sb[:, h * NH + lo:h * NH + lo + 3, :],
                                    q3)
                            else:
                                nc.scalar.copy(qrun[:, gi * 3:gi * 3 + 3, :],
                                               q3)
                        lo = r * 12
                        nc.vector.tensor_mul(
                            ph[:, lo:lo + 9, :],
                            wlb_sb[:, h * NH + lo:h * NH + lo + 9, :],
                            qrun[:])
                    ph4 = ph[:].rearrange("p c (n o) -> p c n o", n=N_NODE)
                    half_tree(nc.vector, ph4, prb, h)
                    uv_ps = ps_f.tile([128, NH * N_NODE], f32,
                                      name=f"uv_ps{h}", tag=f"uv_ps{h}")
                    nc.tensor.matmul(
                        uv_ps[:], f_sb[:],
                        prb[:, h * NH:(h + 1) * NH, :]
                        .rearrange("p c n -> p (c n)"),
                        start=True, stop=True)
                    uv3 = uv_ps[:].rearrange("p (c n) -> p c n", n=N_NODE)
                    hs = slice(h * NH, (h + 1) * NH)
                    if first:
                        nc.scalar.copy(b_sb[:, hs, :], uv3)
                        uvs.append(uv3)
                    else:
                        nc.vector.tensor_add(b_sb[:, hs, :], b_sb[:, hs, :],
                                             uv3)
                        uvs.append(b_sb[:, hs, :])
                return uvs

            def softmax_c(b_srcs):
                e_sb = wp.tile([128, NCH, N_NODE], f32, name="e_sb",
                               tag="e_sb")
                for h in range(2):
                    hs = slice(h * NH, (h + 1) * NH)
                    nc.scalar.activation(e_sb[:, hs, :], b_srcs[h], AF.Exp)
                se = wp.tile([128, NCH], f32, name="se", tag="se")
                nc.vector.reduce_sum(se[:], e_sb[:], axis=AX.X)
                rse = wp.tile([128, NCH], f32, name="rse", tag="rse")
                nc.vector.reciprocal_approx_fast(rse[:], se[:])
                c_sb = wp.tile([128, NCH, N_NODE], bf16, name="c_sb",
                               tag="c_sb")
                nc.vector.tensor_mul(
                    c_sb[:], e_sb[:],
                    rse[:].unsqueeze(2).broadcast_to((128, NCH, N_NODE)))
                return c_sb

            def build_mc(c_sb, wl4_src, dtype, tag):
                """mc = broadcast(c) * wl in 6 chunk-slabs: DVE takes the
                first 4 (PE consumes in chunk order), GpSimd the last 2."""
                mc = wp.tile([128, NCH, NO], dtype, name=tag, tag=tag)
                mc4 = mc[:].rearrange("p c (n o) -> p c n o", n=N_NODE)
                cb = c_sb[:].unsqueeze(3).broadcast_to(
                    (128, NCH, N_NODE, O_SZ))
                for sl in range(6):
                    cs = slice(sl * 12, (sl + 1) * 12)
                    eng = nc.gpsimd if sl in (3, 4) else nc.vector
                    eng.tensor_mul(mc4[:, cs], wl4_src[:, cs], cb[:, cs])
                return mc

            # ---------------- iteration 1 (c uniform = 0.1) ----------------
            s_sb = wp.tile([128, 2, NO], f32, name="s_sb1", tag="s_sb")
            s_matmul_f8(wl8_sb[:], s_sb, scale=0.1 / SW)
            v8 = squash(s_sb, 128, 2, "m", f8, fac_scale=SV)
            uvs = b_update(v8, first=True)

            # ---------------- iteration 2 ----------------
            c_sb = softmax_c(uvs)
            mc8 = build_mc(c_sb, wl84, f8, "mc8")
            s_sb = wp.tile([128, 2, NO], f32, name="s_sb2", tag="s_sb")
            s_matmul_f8(mc8[:], s_sb, scale=1.0 / SW)
            v8 = squash(s_sb, 128, 2, "m", f8, fac_scale=SV)
            uvs = b_update(v8, first=False)

            # ---------------- iteration 3: bf16, own batch shard ----------
            c_sb = softmax_c(uvs)
            mc3 = build_mc(c_sb, wlb4, bf16, "mc3")
            s3_ps = ps_s.tile([B_SH, NO], f32, name="s3_ps", tag="s3_ps")
            for c in range(NCH):
                nc.tensor.matmul(s3_ps[:], xts_sb[:, c, :], mc3[:, c, :],
                                 start=(c == 0), stop=(c == NCH - 1))
            ssh = wp.tile([B_SH, 1, NO], f32, name="ssh", tag="ssh")
            nc.scalar.copy(ssh[:, 0, :], s3_ps[:])
            vsh = squash(ssh, B_SH, 1, "s", f32, newton_iters=2)
            nc.sync.dma_start(y_d[:], vsh[:, 0, :])

    nc.compile()
    return nc


def _pack_pm(arr2d, cols):
    """[J, cols] row-major -> [128, NCH*cols] partition-major contiguous."""
    return np.ascontiguousarray(
        arr2d.reshape(NCH, 128, cols).transpose(1, 0, 2).reshape(
            128, NCH * cols))


def _host_prep(x, W):
    """Per-core input dicts; only xts (the 32-col batch shard of x, bf16)
    differs between cores."""
    import ml_dtypes

    bf = ml_dtypes.bfloat16
    f8 = ml_dtypes.float8_e4m3
    x = np.ascontiguousarray(x, dtype=np.float32)
    W = np.ascontiguousarray(W, dtype=np.float32)
    xt = np.ascontiguousarray(x.transpose(2, 1, 0)).reshape(J, B)
    xik = np.ascontiguousarray(x.transpose(0, 2, 1)).reshape(B, J)
    wl = np.ascontiguousarray(
        (np.float32(0.03) * W[0]).transpose(0, 3, 1, 2)).reshape(J, NO)
    xt8 = _pack_pm(xt.astype(f8), B)
    xik8 = np.ascontiguousarray(
        xik.astype(f8).reshape(2, 128, J).transpose(1, 0, 2).reshape(
            128, 2 * J))
    wl8 = _pack_pm((wl * np.float32(SW)).astype(f8), NO)
    wlb = _pack_pm(wl.astype(bf), NO)
    xtb = xt.astype(bf)
    # F entries 1/(B*SV) = 2^-12: exact in bf16.
    F = (np.kron(np.eye(16, dtype=np.float32),
                 np.ones((8, 8), dtype=np.float32))
         / np.float32(B * SV)).astype(bf)
    base = {"xt8": xt8, "xik8": xik8, "wl8": wl8, "wlb": wlb, "fmat": F}
    in_maps = []
    for c in range(N_CORES):
        m = dict(base)
        m["xts"] = _pack_pm(np.ascontiguousarray(
            xtb[:, c * B_SH:(c + 1) * B_SH]), B_SH)
        in_maps.append(m)
    return in_maps


def _run(in_maps, trace=False, all_cores=False):
    from concourse.bass_utils import run_bass_kernel_spmd

    if "nc" not in _CACHE:
        _CACHE["nc"] = _build_program()
    nc = _CACHE["nc"]
    kwargs = {}
    if all_cores:
        kwargs["trace_cores"] = list(range(N_CORES))
    res = run_bass_kernel_spmd(nc, in_maps, core_ids=list(range(N_CORES)),
                               trace=trace, **kwargs)
    return res


def kernel(x: np.ndarray, W: np.ndarray) -> np.ndarray:
    in_maps = _host_prep(x, W)
    res = _run(in_maps)
    v = np.concatenate([res.results[c]["y"] for c in range(N_CORES)], axis=0)
    return v.reshape(B, N_NODE, O_SZ, 1).astype(np.float32)
